# revision 16
# baseline (speedup 1.0000x reference)
"""Trainium2 Bass kernel for nn_Memory (scatter_memory).

Data-parallel over batch: 8 cores x 8 batches (4096 tokens each).
Math restructure: attn_out = sum_h addr_h @ (mem_value @ out_w_h.T), fusing the
[N,4096]x[4096,512] output projection into tiny per-head [112,512] weights.
Matmul operands are bf16 (fp32 PSUM accumulation); the residual path stays fp32.
rsqrt is computed as Exp(-0.5*Ln(x)) so ScalarE stays on one activation table.
"""

import math
import os
import sys

import numpy as np
import ml_dtypes

sys.path.insert(0, "/opt/trn_rl_repo")

BF16 = ml_dtypes.bfloat16
P = 128
C = 512
H = 8
D = 64
S = 112
RADIUS = 16.0
EPS = 1e-5
NCORES = 8
NLOC = 4096          # tokens per core
TT = 512             # tokens per tile
NCH = TT // P        # chunks per tile
F32 = np.float32

QP_BF16 = os.environ.get("NNMEM_QP_FP32", "") == ""   # proj/sim path in bf16


def _l2n(x, axis):
    n = np.linalg.norm(x, axis=axis, keepdims=True)
    return x / np.maximum(n, 1e-12)


def _chunked(a):
    # [512, X] -> [128, 4, X] with row r = j*128+p -> [p, j, :]
    x = np.ascontiguousarray(a)
    return np.ascontiguousarray(x.reshape(4, P, -1).transpose(1, 0, 2))


def _patch_act_tables():
    """Route every ACT func we emit (Exp/Ln/Identity/Copy) to the single
    combined natural_log_exp_and_others table so ScalarE loads one activation
    table instead of ping-ponging between the exp and ln tables per chunk."""
    from concourse import hw_specs, mybir

    if getattr(hw_specs, "_nnmem_patched", False):
        return
    orig = hw_specs.get_activation_tables
    ours = {
        mybir.ActivationFunctionType.Exp,
        mybir.ActivationFunctionType.Ln,
        mybir.ActivationFunctionType.Identity,
        mybir.ActivationFunctionType.Copy,
    }

    def patched(module_arch):
        t = orig(module_arch)
        if "natural_log_exp_and_others" in t:
            for name, fns in t.items():
                if name != "natural_log_exp_and_others":
                    t[name] = fns - ours
        return t

    hw_specs.get_activation_tables = patched
    hw_specs._nnmem_patched = True
    import concourse.bacc as _bacc

    if getattr(_bacc, "get_activation_tables", None) is orig:
        _bacc.get_activation_tables = patched


def build_nc(ntiles, ln1_triv, ln3_triv, ob_triv):
    import concourse.tile as tile
    from concourse import bacc, mybir
    from concourse.masks import make_identity

    _patch_act_tables()

    fp32 = mybir.dt.float32
    bf16 = mybir.dt.bfloat16
    qpdt = bf16 if QP_BF16 else fp32
    AF = mybir.ActivationFunctionType
    ALU = mybir.AluOpType
    AX = mybir.AxisListType
    LNR = float(math.log(RADIUS))
    LN512 = float(math.log(C))

    ntok = ntiles * TT
    nc = bacc.Bacc("TRN2", target_bir_lowering=False, debug=False)

    # ---- dram params (inputs) ----
    d_qT = nc.declare_dram_parameter("qT", [C, ntok], qpdt, isOutput=False)
    d_q = nc.declare_dram_parameter("q", [ntok, C], fp32, isOutput=False)
    d_vT = nc.declare_dram_parameter("vT", [C, ntok], qpdt, isOutput=False)
    d_wq = nc.declare_dram_parameter("wqT", [P, 4, C], qpdt, isOutput=False)
    d_wv = nc.declare_dram_parameter("wvT", [P, 4, C], qpdt, isOutput=False)
    d_keyn = nc.declare_dram_parameter("keyn", [P, 4, 2 * S], qpdt, isOutput=False)
    d_W = nc.declare_dram_parameter("W", [P, H, C], bf16, isOutput=False)
    d_mv = nc.declare_dram_parameter("mvpad", [P, C], bf16, isOutput=False)
    d_mvT = nc.declare_dram_parameter("mvT", [P, 4, S], qpdt, isOutput=False)
    d_vnT = nc.declare_dram_parameter("vnT", [P, 4, S], qpdt, isOutput=False)
    d_blk = nc.declare_dram_parameter("blk", [P, 4, H], qpdt, isOutput=False)
    d_qb = nc.declare_dram_parameter("qb", [P, 4], fp32, isOutput=False)
    d_vb = nc.declare_dram_parameter("vb", [P, 4], fp32, isOutput=False)
    d_iv = nc.declare_dram_parameter("inv_v", [P, ntok // P], fp32, isOutput=False)
    d_vn32 = nc.declare_dram_parameter("vnT32", [P, 4, S], fp32, isOutput=False)
    d_g1 = d_b1 = d_g3 = d_b3 = d_ob = None
    if not ln1_triv:
        d_g1 = nc.declare_dram_parameter("g1b", [P, C], fp32, isOutput=False)
        d_b1 = nc.declare_dram_parameter("b1b", [P, C], fp32, isOutput=False)
    if not ln3_triv:
        d_g3 = nc.declare_dram_parameter("g3b", [P, C], fp32, isOutput=False)
        d_b3 = nc.declare_dram_parameter("b3b", [P, C], fp32, isOutput=False)
    if not ob_triv:
        d_ob = nc.declare_dram_parameter("obb", [P, C], fp32, isOutput=False)

    # ---- dram outputs ----
    d_fp = nc.declare_dram_parameter("f_pred", [ntok, C], fp32, isOutput=True)
    d_fr = nc.declare_dram_parameter("f_rec", [ntok, C], fp32, isOutput=True)
    d_rl = nc.declare_dram_parameter("recon_out", [1, 1], fp32, isOutput=True)
    d_cl = nc.declare_dram_parameter("contr_out", [1, 1], fp32, isOutput=True)

    fp_t = d_fp[:].rearrange("(kt p) c -> p kt c", p=P)
    fr_t = d_fr[:].rearrange("(kt p) c -> p kt c", p=P)
    qT_t = d_qT[:].rearrange("(j p) n -> p j n", p=P)
    vT_t = d_vT[:].rearrange("(j p) n -> p j n", p=P)
    q_t = d_q[:].rearrange("(kt p) c -> p kt c", p=P)

    with tile.TileContext(nc) as tc:
        with (
            tc.tile_pool(name="singles", bufs=1) as sing,
            tc.tile_pool(name="io", bufs=2) as io,
            tc.tile_pool(name="proj", bufs=2) as proj,
            tc.tile_pool(name="ck", bufs=4) as ck,
            tc.tile_pool(name="psA", bufs=2, space="PSUM") as psA,
            tc.tile_pool(name="psSim", bufs=2, space="PSUM") as psSim,
            tc.tile_pool(name="psTr", bufs=2, space="PSUM") as psTr,
            tc.tile_pool(name="psC", bufs=2, space="PSUM") as psC,
        ):
            # ---------- static setup ----------
            ident = sing.tile([P, P], fp32, tag="ident")
            make_identity(nc, ident)
            identb = sing.tile([P, P], bf16, tag="identb")
            make_identity(nc, identb)
            ones_b = sing.tile([P, 1], bf16, tag="onesb")
            nc.vector.memset(ones_b, 1.0)
            ones_f = sing.tile([P, 1], fp32, tag="onesf")
            nc.vector.memset(ones_f, 1.0)
            eps_sb = sing.tile([P, 1], fp32, tag="eps")
            nc.vector.memset(eps_sb, EPS)
            eps24_sb = sing.tile([P, 1], fp32, tag="eps24")
            nc.vector.memset(eps24_sb, 1e-24)
            lnr_sb = sing.tile([P, 1], fp32, tag="lnr")
            nc.vector.memset(lnr_sb, LNR)
            ln512_sb = sing.tile([P, 1], fp32, tag="ln512")
            nc.vector.memset(ln512_sb, -0.5 * LN512)

            wq_sb = sing.tile([P, 4, C], qpdt, tag="wq")
            nc.sync.dma_start(out=wq_sb, in_=d_wq[:])
            wv_sb = sing.tile([P, 4, C], qpdt, tag="wv")
            nc.sync.dma_start(out=wv_sb, in_=d_wv[:])
            keyn_sb = sing.tile([P, 4, 2 * S], qpdt, tag="keyn")
            nc.sync.dma_start(out=keyn_sb, in_=d_keyn[:])
            W_sb = sing.tile([P, H, C], bf16, tag="W")
            nc.sync.dma_start(out=W_sb, in_=d_W[:])
            mv_sb = sing.tile([P, C], bf16, tag="mv")
            nc.sync.dma_start(out=mv_sb, in_=d_mv[:])
            mvT_sb = sing.tile([P, 4, S], qpdt, tag="mvT")
            nc.sync.dma_start(out=mvT_sb, in_=d_mvT[:])
            vnT_sb = sing.tile([P, 4, S], qpdt, tag="vnT")
            nc.sync.dma_start(out=vnT_sb, in_=d_vnT[:])
            vn32_sb = sing.tile([P, 4, S], fp32, tag="vn32")
            nc.sync.dma_start(out=vn32_sb, in_=d_vn32[:])
            blk_sb = sing.tile([P, 4, H], qpdt, tag="blk")
            nc.sync.dma_start(out=blk_sb, in_=d_blk[:])
            qb_sb = sing.tile([P, 4], fp32, tag="qb")
            nc.sync.dma_start(out=qb_sb, in_=d_qb[:])
            vb_sb = sing.tile([P, 4], fp32, tag="vb")
            nc.sync.dma_start(out=vb_sb, in_=d_vb[:])
            iv_sb = sing.tile([P, ntok // P], fp32, tag="iv")
            nc.sync.dma_start(out=iv_sb, in_=d_iv[:])
            g1_sb = b1_sb = g3_sb = b3_sb = ob_sb = None
            if not ln1_triv:
                g1_sb = sing.tile([P, C], fp32, tag="g1")
                nc.sync.dma_start(out=g1_sb, in_=d_g1[:])
                b1_sb = sing.tile([P, C], fp32, tag="b1")
                nc.sync.dma_start(out=b1_sb, in_=d_b1[:])
            if not ln3_triv:
                g3_sb = sing.tile([P, C], fp32, tag="g3")
                nc.sync.dma_start(out=g3_sb, in_=d_g3[:])
                b3_sb = sing.tile([P, C], fp32, tag="b3")
                nc.sync.dma_start(out=b3_sb, in_=d_b3[:])
            if not ob_triv:
                ob_sb = sing.tile([P, C], fp32, tag="ob")
                nc.sync.dma_start(out=ob_sb, in_=d_ob[:])

            # persistent zero-padded staging tiles (rows S..127 stay zero)
            addrT_pads = []
            for i in range(2):
                t = sing.tile([P, H, P], bf16, tag=f"addrT{i}")
                nc.vector.memset(t, 0.0)
                addrT_pads.append(t)
            addrR_pads = []
            prod_pads = []
            for i in range(2):
                t = sing.tile([P, P], bf16, tag=f"addrR{i}")
                nc.vector.memset(t, 0.0)
                addrR_pads.append(t)
                t2 = sing.tile([P, P], bf16, tag=f"prod{i}")
                nc.vector.memset(t2, 0.0)
                prod_pads.append(t2)

            acc_sb = sing.tile([P, ntok // P], fp32, tag="acc")

            # ---------- contrastive loss (identical on every core) ----------
            negid = sing.tile([P, P], fp32, tag="negid")
            nc.scalar.mul(negid, ident, -1.0)
            g_ps = psA.tile([S, S], fp32, tag="big")
            for j in range(4):
                nc.tensor.matmul(g_ps, vn32_sb[:, j, :], vn32_sb[:, j, :],
                                 start=(j == 0), stop=False)
            nc.tensor.matmul(g_ps, negid[:, :S], ident[:, :S], start=False,
                             stop=True)
            red_pad = sing.tile([P, 1], fp32, tag="redpad")
            nc.vector.memset(red_pad, 0.0)
            nc.vector.tensor_reduce(red_pad[:S, :], g_ps, axis=AX.X, op=ALU.add,
                                    apply_absolute_value=True)
            cl_ps = psC.tile([1, 1], fp32, tag="small")
            nc.tensor.matmul(cl_ps, red_pad, ones_f, start=True, stop=True)
            cl_sb = sing.tile([1, 1], fp32, tag="clsb")
            nc.scalar.mul(cl_sb, cl_ps, 0.01)
            nc.sync.dma_start(out=d_cl[:], in_=cl_sb)

            # ---------- main loop (software-pipelined per chunk) ----------
            def tile_load(t):
                qT_sb = io.tile([P, 4, TT], qpdt, tag="qTin")
                nc.sync.dma_start(out=qT_sb, in_=qT_t[:, :, t * TT:(t + 1) * TT])
                vT_sb = io.tile([P, 4, TT], qpdt, tag="vTin")
                nc.sync.dma_start(out=vT_sb, in_=vT_t[:, :, t * TT:(t + 1) * TT])
                q_sb = io.tile([P, 4, C], fp32, tag="qin")
                nc.sync.dma_start(out=q_sb, in_=q_t[:, t * 4:(t + 1) * 4, :])

                qp_sb = proj.tile([P, 4, TT], qpdt, tag="qp")
                vp_sb = proj.tile([P, 4, TT], qpdt, tag="vp")
                for j in range(4):
                    pq = psA.tile([P, TT], fp32, tag="big")
                    for i in range(4):
                        nc.tensor.matmul(pq, wq_sb[:, i, j * P:(j + 1) * P],
                                         qT_sb[:, i, :], start=(i == 0), stop=(i == 3))
                    nc.scalar.activation(qp_sb[:, j, :], pq, AF.Identity,
                                         bias=qb_sb[:, j:j + 1], scale=1.0)
                for j in range(4):
                    pv = psA.tile([P, TT], fp32, tag="big")
                    for i in range(4):
                        nc.tensor.matmul(pv, wv_sb[:, i, j * P:(j + 1) * P],
                                         vT_sb[:, i, :], start=(i == 0), stop=(i == 3))
                    nc.scalar.activation(vp_sb[:, j, :], pv, AF.Identity,
                                         bias=vb_sb[:, j:j + 1], scale=1.0)

                pmv = psA.tile([S, TT], fp32, tag="big")
                for j in range(4):
                    nc.tensor.matmul(pmv, mvT_sb[:, j, :], vT_sb[:, j, :],
                                     start=(j == 0), stop=(j == 3))
                mvS_sb = io.tile([S, TT], bf16, tag="mvS")
                nc.scalar.copy(mvS_sb, pmv)

                qsq = proj.tile([P, 4, TT], qpdt, tag="qsq")
                nc.vector.tensor_tensor(qsq, qp_sb, qp_sb, ALU.mult)
                vsq = proj.tile([P, 4, TT], qpdt, tag="vsq")
                nc.vector.tensor_tensor(vsq, vp_sb, vp_sb, ALU.mult)
                return dict(q=q_sb, qp=qp_sb, vp=vp_sb, mvS=mvS_sb,
                            qsq=qsq, vsq=vsq)

            def phaseA(ts, k):
                kt = ts["t"] * NCH + k
                ksl = slice(k * P, (k + 1) * P)
                qp_sb, vp_sb = ts["qp"], ts["vp"]

                qsq, vsq = ts["qsq"], ts["vsq"]
                pss = psC.tile([P, 16], fp32, tag="small")
                for j in range(4):
                    nc.tensor.matmul(pss[:, 0:H], qsq[:, j, ksl], blk_sb[:, j, :],
                                     start=(j == 0), stop=(j == 3))
                for j in range(4):
                    nc.tensor.matmul(pss[:, H:H + 1], vsq[:, j, ksl], ones_b,
                                     start=(j == 0), stop=(j == 3))

                lnss = ck.tile([P, H + 1], fp32, tag="lnss")
                nc.scalar.activation(lnss, pss[:, 0:H + 1], AF.Ln,
                                     bias=eps24_sb, scale=1.0)
                rinv = ck.tile([P, H + 1], fp32, tag="rinv")
                nc.scalar.activation(rinv, lnss, AF.Exp, bias=lnr_sb, scale=-0.5)

                simp = psSim.tile([P, 2, 2 * S], fp32, tag="sim")
                simq = psSim.tile([P, 2, 2 * S], fp32, tag="sim")
                for j in range(4):
                    sp = simp if j < 2 else simq
                    nc.tensor.matmul(sp[:, j % 2, :], qp_sb[:, j, ksl],
                                     keyn_sb[:, j, :], start=True, stop=True)

                expv = ck.tile([P, H, S], bf16, tag="expv")
                for h in range(H):
                    sp = simp if h < 4 else simq
                    jj, lh = (h // 2) % 2, h % 2
                    nc.scalar.activation(
                        expv[:, h, :], sp[:, jj, lh * S:(lh + 1) * S], AF.Exp,
                        bias=0.0, scale=rinv[:, h:h + 1])
                sums = ck.tile([P, H], fp32, tag="sums")
                nc.vector.tensor_reduce(sums, expv, axis=AX.X, op=ALU.add)
                nc.vector.reciprocal(sums, sums)
                nc.vector.tensor_tensor(
                    expv, expv, sums[:, :, None].to_broadcast((P, H, S)),
                    ALU.mult)

                pvs = psTr.tile([P, S], fp32, tag="tr")
                for j in range(4):
                    nc.tensor.matmul(pvs, vp_sb[:, j, ksl], vnT_sb[:, j, :],
                                     start=(j == 0), stop=(j == 3))
                rexp = ck.tile([P, S], bf16, tag="rexp")
                nc.scalar.activation(rexp, pvs, AF.Exp, bias=0.0,
                                     scale=rinv[:, H:H + 1])
                rsum = ck.tile([P, 1], fp32, tag="rsum")
                nc.vector.tensor_reduce(rsum, rexp, axis=AX.X, op=ALU.add)
                nc.vector.reciprocal(rsum, rsum)
                nc.vector.tensor_scalar_mul(rexp, rexp, rsum)
                return dict(expv=expv, rexp=rexp)

            def phaseB(ts, k, ph):
                kt = ts["t"] * NCH + k
                ksl = slice(k * P, (k + 1) * P)
                q_sb, mvS_sb = ts["q"], ts["mvS"]
                expv, rexp = ph["expv"], ph["rexp"]
                aT = addrT_pads[kt % 2]
                aR = addrR_pads[kt % 2]
                prd = prod_pads[kt % 2]

                for half in range(2):
                    tp = psTr.tile([S, 4, P], bf16, tag="tr")
                    for hh in range(4):
                        h = half * 4 + hh
                        nc.tensor.transpose(tp[:, hh, :], expv[:, h, :], identb)
                    if half == 0:
                        nc.scalar.copy(aT[:S, 0:4, :], tp)
                    else:
                        nc.vector.tensor_copy(aT[:S, 4:8, :], tp)
                tpr = psTr.tile([S, P], bf16, tag="tr")
                nc.tensor.transpose(tpr, rexp, identb)
                nc.scalar.copy(aR[:S, :], tpr)

                # recon branch first: its LN3 tail overlaps the attn matmuls
                prc = psA.tile([P, C], fp32, tag="big")
                nc.tensor.matmul(prc, aR, mv_sb, start=True, stop=True)

                nc.vector.tensor_tensor(prd[:S, :], aR[:S, :], mvS_sb[:, ksl],
                                        ALU.mult)
                pdot = psC.tile([P, 16], fp32, tag="small")
                nc.tensor.matmul(pdot[:, 0:1], prd, ones_b, start=True,
                                 stop=True)
                dot_sb = ck.tile([P, 1], fp32, tag="dot")
                nc.vector.tensor_copy(dot_sb, pdot[:, 0:1])

                st3 = ck.tile([P, 6], fp32, tag="st3")
                nc.vector.bn_stats(st3, prc)
                mv3 = ck.tile([P, 2], fp32, tag="mv3")
                nc.vector.bn_aggr(mv3, st3)
                vrs = ck.tile([P, 2], fp32, tag="vrs")
                nc.gpsimd.tensor_copy(vrs[:, 0:1], mv3[:, 1:2])
                nc.vector.tensor_scalar(vrs[:, 1:2], mv3[:, 0:1], mv3[:, 0:1],
                                        mv3[:, 1:2], op0=ALU.mult, op1=ALU.add)
                rsb = ck.tile([P, 2], fp32, tag="rsb")
                nc.scalar.activation(rsb, vrs, AF.Ln, bias=eps_sb, scale=1.0)
                nc.scalar.activation(rsb, rsb, AF.Exp, bias=0.0, scale=-0.5)

                # attn matmuls run while LN3 smalls trail on ACT/DVE
                pat = psA.tile([P, C], fp32, tag="big")
                for h in range(H):
                    nc.tensor.matmul(pat, aT[:, h, :], W_sb[:, h, :],
                                     start=(h == 0), stop=(h == H - 1))

                nm3 = ck.tile([P, 1], fp32, tag="nm3")
                nc.vector.tensor_scalar(nm3, mv3[:, 0:1], rsb[:, 0:1], -1.0,
                                        op0=ALU.mult, op1=ALU.mult)
                y_sb = ck.tile([P, C], fp32, tag="y")
                nc.scalar.activation(y_sb, prc, AF.Identity, bias=nm3,
                                     scale=rsb[:, 0:1])
                if not ln3_triv:
                    nc.vector.tensor_tensor(y_sb, y_sb, g3_sb, ALU.mult)
                    nc.vector.tensor_tensor(y_sb, y_sb, b3_sb, ALU.add)

                cosv = ck.tile([P, 1], fp32, tag="cosv")
                nc.vector.tensor_scalar(cosv, dot_sb, rsb[:, 1:2],
                                        iv_sb[:, kt:kt + 1],
                                        op0=ALU.mult, op1=ALU.mult)
                nc.vector.tensor_scalar(acc_sb[:, kt:kt + 1], cosv, -1.0, 1.0,
                                        op0=ALU.mult, op1=ALU.add)

                # ---- LN1 predict ----
                xp = ck.tile([P, C], fp32, tag="xp")
                nc.vector.tensor_tensor(xp, pat, q_sb[:, k, :], ALU.add)
                if not ob_triv:
                    nc.vector.tensor_tensor(xp, xp, ob_sb, ALU.add)
                st1 = ck.tile([P, 6], fp32, tag="st1")
                nc.vector.bn_stats(st1, xp)
                mv1 = ck.tile([P, 2], fp32, tag="mv1")
                nc.vector.bn_aggr(mv1, st1)
                rs1 = ck.tile([P, 1], fp32, tag="rs1")
                nc.scalar.activation(rs1, mv1[:, 1:2], AF.Ln, bias=eps_sb,
                                     scale=1.0)
                nc.scalar.activation(rs1, rs1, AF.Exp, bias=0.0, scale=-0.5)
                nm1 = ck.tile([P, 1], fp32, tag="nm1")
                nc.vector.tensor_scalar(nm1, mv1[:, 0:1], rs1, -1.0,
                                        op0=ALU.mult, op1=ALU.mult)
                fpc = ck.tile([P, C], fp32, tag="fpc")
                nc.vector.tensor_scalar(fpc, xp, rs1, nm1,
                                        op0=ALU.mult, op1=ALU.add)
                if not ln1_triv:
                    nc.vector.tensor_tensor(fpc, fpc, g1_sb, ALU.mult)
                    nc.vector.tensor_tensor(fpc, fpc, b1_sb, ALU.add)
                nc.sync.dma_start(out=fp_t[:, kt, :], in_=fpc)

                # ---- LN1 recon: x2 = q + y ----
                x2 = ck.tile([P, C], fp32, tag="x2")
                nc.vector.tensor_tensor(x2, q_sb[:, k, :], y_sb, ALU.add)
                st2 = ck.tile([P, 6], fp32, tag="st2")
                nc.vector.bn_stats(st2, x2)
                mv2 = ck.tile([P, 2], fp32, tag="mv2")
                nc.vector.bn_aggr(mv2, st2)
                rs2 = ck.tile([P, 1], fp32, tag="rs2")
                nc.scalar.activation(rs2, mv2[:, 1:2], AF.Ln, bias=eps_sb,
                                     scale=1.0)
                nc.scalar.activation(rs2, rs2, AF.Exp, bias=0.0, scale=-0.5)
                nm2 = ck.tile([P, 1], fp32, tag="nm2")
                nc.vector.tensor_scalar(nm2, mv2[:, 0:1], rs2, -1.0,
                                        op0=ALU.mult, op1=ALU.mult)
                frc = ck.tile([P, C], fp32, tag="frc")
                nc.vector.tensor_scalar(frc, x2, rs2, nm2,
                                        op0=ALU.mult, op1=ALU.add)
                if not ln1_triv:
                    nc.vector.tensor_tensor(frc, frc, g1_sb, ALU.mult)
                    nc.vector.tensor_tensor(frc, frc, b1_sb, ALU.add)
                nc.sync.dma_start(out=fr_t[:, kt, :], in_=frc)

            # pipeline: A(k+1) issued before B(k)
            pend = None  # (ts, k, ph)
            for t in range(ntiles):
                ts = tile_load(t)
                ts["t"] = t
                for k in range(NCH):
                    ph = phaseA(ts, k)
                    if pend is not None:
                        phaseB(*pend)
                    pend = (ts, k, ph)
            phaseB(*pend)

            # ---------- recon partial: sum |1-cos| over this core ----------
            accr = sing.tile([P, 1], fp32, tag="accr")
            nc.vector.tensor_reduce(accr, acc_sb, axis=AX.X, op=ALU.add,
                                    apply_absolute_value=True)
            prl = psC.tile([1, 1], fp32, tag="small")
            nc.tensor.matmul(prl, accr, ones_f, start=True, stop=True)
            rl_sb = sing.tile([1, 1], fp32, tag="rlsb")
            nc.scalar.copy(rl_sb, prl)
            nc.sync.dma_start(out=d_rl[:], in_=rl_sb)

    nc.compile()
    return nc


_NC_CACHE = {}
LAST_RESULTS = None


def _get_nc(key):
    if key not in _NC_CACHE:
        _NC_CACHE[key] = build_nc(*key)
    return _NC_CACHE[key]


def prep_shared(mem_key, mem_value, q_w, q_b, v_w, v_b, out_w, out_b,
                ln1_g, ln1_b, ln3_g, ln3_b):
    f64 = np.float64
    qpdt = BF16 if QP_BF16 else F32
    key_n = _l2n(mem_key.astype(f64).reshape(H, S, D), 2)      # [8,112,64]
    keyn_blk = np.zeros((P, 4, 2 * S), f64)
    for j in range(4):
        for l in range(2):
            h = 2 * j + l
            keyn_blk[l * D:(l + 1) * D, j, l * S:(l + 1) * S] = key_n[h].T
    mvf = mem_value.astype(f64)                                 # [112,512]
    W = np.zeros((P, H, C), f64)
    for h in range(H):
        W[:S, h, :] = mvf @ out_w.astype(f64)[:, h * C:(h + 1) * C].T
    vn = _l2n(mvf, 1)                                           # [112,512]
    mv_pad = np.zeros((P, C), f64)
    mv_pad[:S] = mvf
    blk = np.zeros((P, 4, H), f64)
    for j in range(4):
        for i in range(P):
            blk[i, j, (j * P + i) // D] = 1.0

    def cvt(x, dt):
        return np.ascontiguousarray(np.asarray(x).astype(dt))

    shared = {
        "wqT": cvt(_chunked(q_w.astype(f64).T), qpdt),
        "wvT": cvt(_chunked(v_w.astype(f64).T), qpdt),
        "keyn": cvt(keyn_blk, qpdt),
        "W": cvt(W, BF16),
        "mvpad": cvt(mv_pad, BF16),
        "mvT": cvt(_chunked(mvf.T), qpdt),
        "vnT": cvt(_chunked(vn.T), qpdt),
        "vnT32": cvt(_chunked(vn.T), F32),
        "blk": cvt(blk, qpdt),
        "qb": cvt(q_b.reshape(4, P).T, F32),
        "vb": cvt(v_b.reshape(4, P).T, F32),
    }
    ln1_triv = bool(np.all(ln1_g == 1.0) and np.all(ln1_b == 0.0))
    ln3_triv = bool(np.all(ln3_g == 1.0) and np.all(ln3_b == 0.0))
    ob_triv = bool(np.all(out_b == 0.0))
    if not ln1_triv:
        shared["g1b"] = cvt(np.tile(ln1_g[None, :], (P, 1)), F32)
        shared["b1b"] = cvt(np.tile(ln1_b[None, :], (P, 1)), F32)
    if not ln3_triv:
        shared["g3b"] = cvt(np.tile(ln3_g[None, :], (P, 1)), F32)
        shared["b3b"] = cvt(np.tile(ln3_b[None, :], (P, 1)), F32)
    if not ob_triv:
        shared["obb"] = cvt(np.tile(out_b[None, :], (P, 1)), F32)
    return shared, (ln1_triv, ln3_triv, ob_triv)


def prep_core(q_i, v_i, ntok):
    # q_i, v_i: [ntok, 512] float32
    qpdt = BF16 if QP_BF16 else F32
    iv = 1.0 / np.maximum(np.linalg.norm(v_i.astype(np.float64), axis=1), 1e-12)
    iv = iv / np.sqrt(512.0)
    return {
        "qT": np.ascontiguousarray(q_i.T.astype(qpdt)),
        "q": np.ascontiguousarray(q_i),
        "vT": np.ascontiguousarray(v_i.T.astype(qpdt)),
        "inv_v": np.ascontiguousarray(iv.reshape(ntok // P, P).T.astype(F32)),
    }


def kernel(query, value, mem_key, mem_value, q_w, q_b, v_w, v_b,
           out_w, out_b, ln1_g, ln1_b, ln3_g, ln3_b):
    from concourse.bass_utils import run_bass_kernel_spmd

    query = np.asarray(query, F32)
    value = np.asarray(value, F32)
    B, SEQ, _ = query.shape
    ntok_total = B * SEQ
    ntok = ntok_total // NCORES
    ntiles = ntok // TT

    shared, flags = prep_shared(
        np.asarray(mem_key, F32), np.asarray(mem_value, F32),
        np.asarray(q_w, F32), np.asarray(q_b, F32),
        np.asarray(v_w, F32), np.asarray(v_b, F32),
        np.asarray(out_w, F32), np.asarray(out_b, F32),
        np.asarray(ln1_g, F32), np.asarray(ln1_b, F32),
        np.asarray(ln3_g, F32), np.asarray(ln3_b, F32))

    nc = _get_nc((ntiles,) + flags)

    qs = query.reshape(NCORES, ntok, C)
    vs = value.reshape(NCORES, ntok, C)
    in_maps = []
    for i in range(NCORES):
        m = dict(shared)
        m.update(prep_core(qs[i], vs[i], ntok))
        in_maps.append(m)

    res = run_bass_kernel_spmd(nc, in_maps, list(range(NCORES)))
    global LAST_RESULTS
    LAST_RESULTS = res
    outs = res.results

    f_pred = np.concatenate([outs[i]["f_pred"] for i in range(NCORES)], axis=0)
    f_rec = np.concatenate([outs[i]["f_rec"] for i in range(NCORES)], axis=0)
    recon = sum(float(outs[i]["recon_out"][0, 0]) for i in range(NCORES))
    recon = np.asarray(recon / ntok_total, F32)
    contr = np.asarray(float(outs[0]["contr_out"][0, 0]), F32)
    return (f_pred.reshape(B, SEQ, C), f_rec.reshape(B, SEQ, C), recon, contr)


# revision 19
# speedup vs baseline: 1.1714x; 1.1714x over previous
"""Trainium2 Bass kernel for nn_Memory (scatter_memory).

Data-parallel over batch: 8 cores x 8 batches (4096 tokens each).
Math restructure: attn_out = sum_h addr_h @ (mem_value @ out_w_h.T), fusing the
[N,4096]x[4096,512] output projection into tiny per-head [112,512] weights.
Matmul operands are bf16 (fp32 PSUM accumulation); the residual path stays fp32.
rsqrt is computed as Exp(-0.5*Ln(x)) so ScalarE stays on one activation table.
"""

import math
import os
import sys

import numpy as np
import ml_dtypes

sys.path.insert(0, "/opt/trn_rl_repo")

BF16 = ml_dtypes.bfloat16
P = 128
C = 512
H = 8
D = 64
S = 112
RADIUS = 16.0
EPS = 1e-5
NCORES = 8
NLOC = 4096          # tokens per core
TT = 512             # tokens per tile
NCH = TT // P        # chunks per tile
F32 = np.float32

QP_BF16 = os.environ.get("NNMEM_QP_FP32", "") == ""   # proj/sim path in bf16


def _l2n(x, axis):
    n = np.linalg.norm(x, axis=axis, keepdims=True)
    return x / np.maximum(n, 1e-12)


def _chunked(a):
    # [512, X] -> [128, 4, X] with row r = j*128+p -> [p, j, :]
    x = np.ascontiguousarray(a)
    return np.ascontiguousarray(x.reshape(4, P, -1).transpose(1, 0, 2))


def _patch_act_tables():
    """Route every ACT func we emit (Exp/Ln/Identity/Copy) to the single
    combined natural_log_exp_and_others table so ScalarE loads one activation
    table instead of ping-ponging between the exp and ln tables per chunk."""
    from concourse import hw_specs, mybir

    if getattr(hw_specs, "_nnmem_patched", False):
        return
    orig = hw_specs.get_activation_tables
    ours = {
        mybir.ActivationFunctionType.Exp,
        mybir.ActivationFunctionType.Ln,
        mybir.ActivationFunctionType.Identity,
        mybir.ActivationFunctionType.Copy,
    }

    def patched(module_arch):
        t = orig(module_arch)
        if "natural_log_exp_and_others" in t:
            for name, fns in t.items():
                if name != "natural_log_exp_and_others":
                    t[name] = fns - ours
        return t

    hw_specs.get_activation_tables = patched
    hw_specs._nnmem_patched = True
    import concourse.bacc as _bacc

    if getattr(_bacc, "get_activation_tables", None) is orig:
        _bacc.get_activation_tables = patched


def build_nc(ntiles, ln1_triv, ln3_triv, ob_triv):
    import concourse.tile as tile
    from concourse import bacc, mybir
    from concourse.masks import make_identity

    _patch_act_tables()

    fp32 = mybir.dt.float32
    bf16 = mybir.dt.bfloat16
    qpdt = bf16 if QP_BF16 else fp32
    AF = mybir.ActivationFunctionType
    ALU = mybir.AluOpType
    AX = mybir.AxisListType
    LNR = float(math.log(RADIUS))
    LN512 = float(math.log(C))

    ntok = ntiles * TT
    nc = bacc.Bacc("TRN2", target_bir_lowering=False, debug=False)

    # ---- dram params (inputs) ----
    d_qT = nc.declare_dram_parameter("qT", [C, ntok], qpdt, isOutput=False)
    d_q = nc.declare_dram_parameter("q", [ntok, C], fp32, isOutput=False)
    d_vT = nc.declare_dram_parameter("vT", [C, ntok], qpdt, isOutput=False)
    d_wq = nc.declare_dram_parameter("wqT", [P, 4, C], qpdt, isOutput=False)
    d_wv = nc.declare_dram_parameter("wvT", [P, 4, C], qpdt, isOutput=False)
    d_keyn = nc.declare_dram_parameter("keyn", [P, 4, 2 * S], qpdt, isOutput=False)
    d_W = nc.declare_dram_parameter("W", [P, H, C], bf16, isOutput=False)
    d_mv = nc.declare_dram_parameter("mvpad", [P, C], bf16, isOutput=False)
    d_mvT = nc.declare_dram_parameter("mvT", [P, 4, S], qpdt, isOutput=False)
    d_vnT = nc.declare_dram_parameter("vnT", [P, 4, S], qpdt, isOutput=False)
    d_blk = nc.declare_dram_parameter("blk", [P, 4, H], qpdt, isOutput=False)
    d_qb = nc.declare_dram_parameter("qb", [P, 4], fp32, isOutput=False)
    d_vb = nc.declare_dram_parameter("vb", [P, 4], fp32, isOutput=False)
    d_iv = nc.declare_dram_parameter("inv_v", [P, ntok // P], fp32, isOutput=False)
    d_vn32 = nc.declare_dram_parameter("vnT32", [P, 4, S], fp32, isOutput=False)
    d_g1 = d_b1 = d_g3 = d_b3 = d_ob = None
    if not ln1_triv:
        d_g1 = nc.declare_dram_parameter("g1b", [P, C], fp32, isOutput=False)
        d_b1 = nc.declare_dram_parameter("b1b", [P, C], fp32, isOutput=False)
    if not ln3_triv:
        d_g3 = nc.declare_dram_parameter("g3b", [P, C], fp32, isOutput=False)
        d_b3 = nc.declare_dram_parameter("b3b", [P, C], fp32, isOutput=False)
    if not ob_triv:
        d_ob = nc.declare_dram_parameter("obb", [P, C], fp32, isOutput=False)

    # ---- dram outputs ----
    d_fp = nc.declare_dram_parameter("f_pred", [ntok, C], fp32, isOutput=True)
    d_fr = nc.declare_dram_parameter("f_rec", [ntok, C], fp32, isOutput=True)
    d_rl = nc.declare_dram_parameter("recon_out", [1, 1], fp32, isOutput=True)
    d_cl = nc.declare_dram_parameter("contr_out", [1, 1], fp32, isOutput=True)

    fp_t = d_fp[:].rearrange("(kt p) c -> p kt c", p=P)
    fr_t = d_fr[:].rearrange("(kt p) c -> p kt c", p=P)
    qT_t = d_qT[:].rearrange("(j p) n -> p j n", p=P)
    vT_t = d_vT[:].rearrange("(j p) n -> p j n", p=P)
    q_t = d_q[:].rearrange("(kt p) c -> p kt c", p=P)

    with tile.TileContext(nc) as tc:
        with (
            tc.tile_pool(name="singles", bufs=1) as sing,
            tc.tile_pool(name="io", bufs=2) as io,
            tc.tile_pool(name="proj", bufs=2) as proj,
            tc.tile_pool(name="ck", bufs=3) as ck,
            tc.tile_pool(name="psA", bufs=2, space="PSUM") as psA,
            tc.tile_pool(name="psSim", bufs=1, space="PSUM") as psSim,
            tc.tile_pool(name="psTr", bufs=2, space="PSUM") as psTr,
            tc.tile_pool(name="psC", bufs=1, space="PSUM") as psC,
        ):
            # ---------- static setup ----------
            ident = sing.tile([P, P], fp32, tag="ident")
            make_identity(nc, ident)
            identb = sing.tile([P, P], bf16, tag="identb")
            make_identity(nc, identb)
            ones_b = sing.tile([P, 1], bf16, tag="onesb")
            nc.vector.memset(ones_b, 1.0)
            ones_f = sing.tile([P, 1], fp32, tag="onesf")
            nc.vector.memset(ones_f, 1.0)
            eps_sb = sing.tile([P, 1], fp32, tag="eps")
            nc.vector.memset(eps_sb, EPS)
            eps24_sb = sing.tile([P, 1], fp32, tag="eps24")
            nc.vector.memset(eps24_sb, 1e-24)
            lnr_sb = sing.tile([P, 1], fp32, tag="lnr")
            nc.vector.memset(lnr_sb, LNR)
            ln512_sb = sing.tile([P, 1], fp32, tag="ln512")
            nc.vector.memset(ln512_sb, -0.5 * LN512)

            wq_sb = sing.tile([P, 4, C], qpdt, tag="wq")
            nc.sync.dma_start(out=wq_sb, in_=d_wq[:])
            wv_sb = sing.tile([P, 4, C], qpdt, tag="wv")
            nc.sync.dma_start(out=wv_sb, in_=d_wv[:])
            keyn_sb = sing.tile([P, 4, 2 * S], qpdt, tag="keyn")
            nc.sync.dma_start(out=keyn_sb, in_=d_keyn[:])
            W_sb = sing.tile([P, H, C], bf16, tag="W")
            nc.sync.dma_start(out=W_sb, in_=d_W[:])
            mv_sb = sing.tile([P, C], bf16, tag="mv")
            nc.sync.dma_start(out=mv_sb, in_=d_mv[:])
            mvT_sb = sing.tile([P, 4, S], qpdt, tag="mvT")
            nc.sync.dma_start(out=mvT_sb, in_=d_mvT[:])
            vnT_sb = sing.tile([P, 4, S], qpdt, tag="vnT")
            nc.sync.dma_start(out=vnT_sb, in_=d_vnT[:])
            vn32_sb = sing.tile([P, 4, S], fp32, tag="vn32")
            nc.sync.dma_start(out=vn32_sb, in_=d_vn32[:])
            blk_sb = sing.tile([P, 4, H], qpdt, tag="blk")
            nc.sync.dma_start(out=blk_sb, in_=d_blk[:])
            qb_sb = sing.tile([P, 4], fp32, tag="qb")
            nc.sync.dma_start(out=qb_sb, in_=d_qb[:])
            vb_sb = sing.tile([P, 4], fp32, tag="vb")
            nc.sync.dma_start(out=vb_sb, in_=d_vb[:])
            iv_sb = sing.tile([P, ntok // P], fp32, tag="iv")
            nc.sync.dma_start(out=iv_sb, in_=d_iv[:])
            g1_sb = b1_sb = g3_sb = b3_sb = ob_sb = None
            if not ln1_triv:
                g1_sb = sing.tile([P, C], fp32, tag="g1")
                nc.sync.dma_start(out=g1_sb, in_=d_g1[:])
                b1_sb = sing.tile([P, C], fp32, tag="b1")
                nc.sync.dma_start(out=b1_sb, in_=d_b1[:])
            if not ln3_triv:
                g3_sb = sing.tile([P, C], fp32, tag="g3")
                nc.sync.dma_start(out=g3_sb, in_=d_g3[:])
                b3_sb = sing.tile([P, C], fp32, tag="b3")
                nc.sync.dma_start(out=b3_sb, in_=d_b3[:])
            if not ob_triv:
                ob_sb = sing.tile([P, C], fp32, tag="ob")
                nc.sync.dma_start(out=ob_sb, in_=d_ob[:])

            # persistent zero-padded staging tiles (rows S..127 stay zero)
            addrT_pads = []
            prod_pads = []
            for i in range(2):
                t = sing.tile([P, H + 1, P], bf16, tag=f"addrT{i}")
                nc.vector.memset(t, 0.0)
                addrT_pads.append(t)
                t2 = sing.tile([P, P], bf16, tag=f"prod{i}")
                nc.vector.memset(t2, 0.0)
                prod_pads.append(t2)

            acc_sb = sing.tile([P, ntok // P], fp32, tag="acc")

            # ---------- contrastive loss (identical on every core) ----------
            negid = sing.tile([P, P], fp32, tag="negid")
            nc.scalar.mul(negid, ident, -1.0)
            g_ps = psA.tile([S, S], fp32, tag="big")
            for j in range(4):
                nc.tensor.matmul(g_ps, vn32_sb[:, j, :], vn32_sb[:, j, :],
                                 start=(j == 0), stop=False)
            nc.tensor.matmul(g_ps, negid[:, :S], ident[:, :S], start=False,
                             stop=True)
            red_pad = sing.tile([P, 1], fp32, tag="redpad")
            nc.vector.memset(red_pad, 0.0)
            nc.vector.tensor_reduce(red_pad[:S, :], g_ps, axis=AX.X, op=ALU.add,
                                    apply_absolute_value=True)
            cl_ps = psC.tile([1, 1], fp32, tag="small")
            nc.tensor.matmul(cl_ps, red_pad, ones_f, start=True, stop=True)
            cl_sb = sing.tile([1, 1], fp32, tag="clsb")
            nc.scalar.mul(cl_sb, cl_ps, 0.01)
            nc.sync.dma_start(out=d_cl[:], in_=cl_sb)

            # ---------- main loop (software-pipelined per chunk) ----------
            def tile_load(t):
                qT_sb = io.tile([P, 4, TT], qpdt, tag="qTin")
                nc.sync.dma_start(out=qT_sb, in_=qT_t[:, :, t * TT:(t + 1) * TT])
                vT_sb = io.tile([P, 4, TT], qpdt, tag="vTin")
                nc.sync.dma_start(out=vT_sb, in_=vT_t[:, :, t * TT:(t + 1) * TT])
                q_sb = io.tile([P, 4, C], fp32, tag="qin")
                nc.sync.dma_start(out=q_sb, in_=q_t[:, t * 4:(t + 1) * 4, :])

                qp_sb = proj.tile([P, 4, TT], qpdt, tag="qp")
                vp_sb = proj.tile([P, 4, TT], qpdt, tag="vp")
                for j in range(4):
                    pq = psA.tile([P, TT], fp32, tag="big")
                    for i in range(4):
                        nc.tensor.matmul(pq, wq_sb[:, i, j * P:(j + 1) * P],
                                         qT_sb[:, i, :], start=(i == 0), stop=(i == 3))
                    nc.scalar.activation(qp_sb[:, j, :], pq, AF.Identity,
                                         bias=qb_sb[:, j:j + 1], scale=1.0)
                for j in range(4):
                    pv = psA.tile([P, TT], fp32, tag="big")
                    for i in range(4):
                        nc.tensor.matmul(pv, wv_sb[:, i, j * P:(j + 1) * P],
                                         vT_sb[:, i, :], start=(i == 0), stop=(i == 3))
                    nc.scalar.activation(vp_sb[:, j, :], pv, AF.Identity,
                                         bias=vb_sb[:, j:j + 1], scale=1.0)

                pmv = psA.tile([S, TT], fp32, tag="big")
                for j in range(4):
                    nc.tensor.matmul(pmv, mvT_sb[:, j, :], vT_sb[:, j, :],
                                     start=(j == 0), stop=(j == 3))
                mvS_sb = io.tile([S, TT], bf16, tag="mvS")
                nc.scalar.copy(mvS_sb, pmv)

                qsq = proj.tile([P, 4, TT], qpdt, tag="qsq")
                nc.vector.tensor_tensor(qsq, qp_sb, qp_sb, ALU.mult)
                vsq = proj.tile([P, 4, TT], qpdt, tag="vsq")
                nc.vector.tensor_tensor(vsq, vp_sb, vp_sb, ALU.mult)

                # norms for all 4 chunks at tile level
                pss4 = psC.tile([P, 4, 16], fp32, tag="small")
                for k4 in range(NCH):
                    ks4 = slice(k4 * P, (k4 + 1) * P)
                    for j in range(4):
                        nc.tensor.matmul(pss4[:, k4, 0:H], qsq[:, j, ks4],
                                         blk_sb[:, j, :],
                                         start=(j == 0), stop=(j == 3))
                    for j in range(4):
                        nc.tensor.matmul(pss4[:, k4, H:H + 1], vsq[:, j, ks4],
                                         ones_b, start=(j == 0), stop=(j == 3))
                lnss4 = io.tile([P, 4, H + 1], fp32, tag="lnss4")
                nc.scalar.activation(lnss4, pss4[:, :, 0:H + 1], AF.Ln,
                                     bias=eps24_sb, scale=1.0)
                rinv4 = io.tile([P, 4, H + 1], fp32, tag="rinv4")
                nc.scalar.activation(rinv4, lnss4, AF.Exp, bias=lnr_sb,
                                     scale=-0.5)

                # recon sims for all 4 chunks (dot results go in col 112:113)
                pvs4 = psC.tile([P, 4, P], fp32, tag="small2")
                for k4 in range(NCH):
                    ks4 = slice(k4 * P, (k4 + 1) * P)
                    for j in range(4):
                        nc.tensor.matmul(pvs4[:, k4, 0:S], vp_sb[:, j, ks4],
                                         vnT_sb[:, j, :],
                                         start=(j == 0), stop=(j == 3))
                return dict(q=q_sb, qp=qp_sb, vp=vp_sb, mvS=mvS_sb,
                            rinv4=rinv4, pvs4=pvs4)

            def phaseA(ts, k):
                kt = ts["t"] * NCH + k
                ksl = slice(k * P, (k + 1) * P)
                qp_sb = ts["qp"]
                rinv4, pvs4 = ts["rinv4"], ts["pvs4"]

                simp = psSim.tile([P, 4, 256], fp32, tag="sim")
                for j in range(4):
                    nc.tensor.matmul(simp[:, j, 0:2 * S], qp_sb[:, j, ksl],
                                     keyn_sb[:, j, :], start=True, stop=True)

                # prescale all 8 heads + recon into one [128, 9, 112] bf16 tile
                es = ck.tile([P, H + 1, S], fp32, tag="es")
                sim4d = simp[:, :, 0:2 * S].rearrange("p j (l s) -> p j l s", l=2)
                nc.vector.tensor_tensor(
                    es[:, 0:H, :].rearrange("p (j l) s -> p j l s", l=2),
                    sim4d,
                    rinv4[:, k, 0:H].rearrange("p (j l) -> p j l", l=2)[:, :, :, None]
                    .to_broadcast((P, 4, 2, S)),
                    ALU.mult)
                nc.vector.tensor_scalar_mul(es[:, H, :], pvs4[:, k, 0:S],
                                            rinv4[:, k, H:H + 1])

                expv = ck.tile([P, H + 1, S], bf16, tag="expv")
                nc.scalar.activation(expv, es, AF.Exp, bias=0.0, scale=1.0)
                sums = ck.tile([P, H + 1], fp32, tag="sums")
                nc.vector.tensor_reduce(sums, expv, axis=AX.X, op=ALU.add)
                nc.vector.reciprocal(sums, sums)
                nc.vector.tensor_tensor(
                    expv, expv, sums[:, :, None].to_broadcast((P, H + 1, S)),
                    ALU.mult)
                return dict(expv=expv)

            def phaseB(ts, k, ph):
                kt = ts["t"] * NCH + k
                ksl = slice(k * P, (k + 1) * P)
                q_sb, mvS_sb = ts["q"], ts["mvS"]
                pvs4 = ts["pvs4"]
                expv = ph["expv"]
                aT = addrT_pads[kt % 2]
                prd = prod_pads[kt % 2]

                for half in range(2):
                    tp = psTr.tile([S, 4, P], bf16, tag="tr")
                    for hh in range(4):
                        h = half * 4 + hh
                        nc.tensor.transpose(tp[:, hh, :], expv[:, h, :], identb)
                    if half == 0:
                        nc.scalar.copy(aT[:S, 0:4, :], tp)
                    else:
                        nc.vector.tensor_copy(aT[:S, 4:8, :], tp)
                tpr = psTr.tile([S, P], bf16, tag="tr")
                nc.tensor.transpose(tpr, expv[:, H, :], identb)
                nc.scalar.copy(aT[:S, H, :], tpr)

                # recon branch first: its LN3 tail overlaps the attn matmuls
                prc = psA.tile([P, C], fp32, tag="big")
                nc.tensor.matmul(prc, aT[:, H, :], mv_sb, start=True, stop=True)

                nc.vector.tensor_tensor(prd[:S, :], aT[:S, H, :], mvS_sb[:, ksl],
                                        ALU.mult)
                nc.tensor.matmul(pvs4[:, k, S:S + 1], prd, ones_b, start=True,
                                 stop=True)
                dot_sb = ck.tile([P, 1], fp32, tag="dot")
                nc.vector.tensor_copy(dot_sb, pvs4[:, k, S:S + 1])

                st3 = ck.tile([P, 6], fp32, tag="st3")
                nc.vector.bn_stats(st3, prc)
                mv3 = ck.tile([P, 2], fp32, tag="mv3")
                nc.vector.bn_aggr(mv3, st3)
                vrs = ck.tile([P, 2], fp32, tag="vrs")
                nc.gpsimd.tensor_copy(vrs[:, 0:1], mv3[:, 1:2])
                nc.vector.tensor_scalar(vrs[:, 1:2], mv3[:, 0:1], mv3[:, 0:1],
                                        mv3[:, 1:2], op0=ALU.mult, op1=ALU.add)
                rsb = ck.tile([P, 2], fp32, tag="rsb")
                nc.scalar.activation(rsb, vrs, AF.Ln, bias=eps_sb, scale=1.0)
                nc.scalar.activation(rsb, rsb, AF.Exp, bias=0.0, scale=-0.5)

                # attn matmuls run while LN3 smalls trail on ACT/DVE
                pat = psA.tile([P, C], fp32, tag="big")
                for h in range(H):
                    nc.tensor.matmul(pat, aT[:, h, :], W_sb[:, h, :],
                                     start=(h == 0), stop=(h == H - 1))

                nm3 = ck.tile([P, 1], fp32, tag="nm3")
                nc.vector.tensor_scalar(nm3, mv3[:, 0:1], rsb[:, 0:1], -1.0,
                                        op0=ALU.mult, op1=ALU.mult)
                y_sb = ck.tile([P, C], fp32, tag="y")
                nc.scalar.activation(y_sb, prc, AF.Identity, bias=nm3,
                                     scale=rsb[:, 0:1])
                if not ln3_triv:
                    nc.vector.tensor_tensor(y_sb, y_sb, g3_sb, ALU.mult)
                    nc.vector.tensor_tensor(y_sb, y_sb, b3_sb, ALU.add)

                cosv = ck.tile([P, 1], fp32, tag="cosv")
                nc.vector.tensor_scalar(cosv, dot_sb, rsb[:, 1:2],
                                        iv_sb[:, kt:kt + 1],
                                        op0=ALU.mult, op1=ALU.mult)
                nc.vector.tensor_scalar(acc_sb[:, kt:kt + 1], cosv, -1.0, 1.0,
                                        op0=ALU.mult, op1=ALU.add)

                # ---- LN1 predict ----
                xp = ck.tile([P, C], fp32, tag="xp")
                nc.vector.tensor_tensor(xp, pat, q_sb[:, k, :], ALU.add)
                if not ob_triv:
                    nc.vector.tensor_tensor(xp, xp, ob_sb, ALU.add)
                st1 = ck.tile([P, 6], fp32, tag="st1")
                nc.vector.bn_stats(st1, xp)
                mv1 = ck.tile([P, 2], fp32, tag="mv1")
                nc.vector.bn_aggr(mv1, st1)
                rs1 = ck.tile([P, 1], fp32, tag="rs1")
                nc.scalar.activation(rs1, mv1[:, 1:2], AF.Ln, bias=eps_sb,
                                     scale=1.0)
                nc.scalar.activation(rs1, rs1, AF.Exp, bias=0.0, scale=-0.5)
                nm1 = ck.tile([P, 1], fp32, tag="nm1")
                nc.vector.tensor_scalar(nm1, mv1[:, 0:1], rs1, -1.0,
                                        op0=ALU.mult, op1=ALU.mult)
                fpc = ck.tile([P, C], fp32, tag="fpc")
                nc.scalar.activation(fpc, xp, AF.Identity, bias=nm1, scale=rs1)
                if not ln1_triv:
                    nc.vector.tensor_tensor(fpc, fpc, g1_sb, ALU.mult)
                    nc.vector.tensor_tensor(fpc, fpc, b1_sb, ALU.add)
                nc.sync.dma_start(out=fp_t[:, kt, :], in_=fpc)

                # ---- LN1 recon: x2 = q + y ----
                x2 = ck.tile([P, C], fp32, tag="x2")
                nc.vector.tensor_tensor(x2, q_sb[:, k, :], y_sb, ALU.add)
                st2 = ck.tile([P, 6], fp32, tag="st2")
                nc.vector.bn_stats(st2, x2)
                mv2 = ck.tile([P, 2], fp32, tag="mv2")
                nc.vector.bn_aggr(mv2, st2)
                rs2 = ck.tile([P, 1], fp32, tag="rs2")
                nc.scalar.activation(rs2, mv2[:, 1:2], AF.Ln, bias=eps_sb,
                                     scale=1.0)
                nc.scalar.activation(rs2, rs2, AF.Exp, bias=0.0, scale=-0.5)
                nm2 = ck.tile([P, 1], fp32, tag="nm2")
                nc.vector.tensor_scalar(nm2, mv2[:, 0:1], rs2, -1.0,
                                        op0=ALU.mult, op1=ALU.mult)
                frc = ck.tile([P, C], fp32, tag="frc")
                nc.scalar.activation(frc, x2, AF.Identity, bias=nm2, scale=rs2)
                if not ln1_triv:
                    nc.vector.tensor_tensor(frc, frc, g1_sb, ALU.mult)
                    nc.vector.tensor_tensor(frc, frc, b1_sb, ALU.add)
                nc.sync.dma_start(out=fr_t[:, kt, :], in_=frc)

            # pipeline: A(k+1) issued before B(k)
            pend = None  # (ts, k, ph)
            for t in range(ntiles):
                ts = tile_load(t)
                ts["t"] = t
                for k in range(NCH):
                    ph = phaseA(ts, k)
                    if pend is not None:
                        phaseB(*pend)
                    pend = (ts, k, ph)
            phaseB(*pend)

            # ---------- recon partial: sum |1-cos| over this core ----------
            accr = sing.tile([P, 1], fp32, tag="accr")
            nc.vector.tensor_reduce(accr, acc_sb, axis=AX.X, op=ALU.add,
                                    apply_absolute_value=True)
            prl = psC.tile([1, 1], fp32, tag="small")
            nc.tensor.matmul(prl, accr, ones_f, start=True, stop=True)
            rl_sb = sing.tile([1, 1], fp32, tag="rlsb")
            nc.scalar.copy(rl_sb, prl)
            nc.sync.dma_start(out=d_rl[:], in_=rl_sb)

    nc.compile()
    return nc


_NC_CACHE = {}
LAST_RESULTS = None


def _get_nc(key):
    if key not in _NC_CACHE:
        _NC_CACHE[key] = build_nc(*key)
    return _NC_CACHE[key]


def prep_shared(mem_key, mem_value, q_w, q_b, v_w, v_b, out_w, out_b,
                ln1_g, ln1_b, ln3_g, ln3_b):
    f64 = np.float64
    qpdt = BF16 if QP_BF16 else F32
    key_n = _l2n(mem_key.astype(f64).reshape(H, S, D), 2)      # [8,112,64]
    keyn_blk = np.zeros((P, 4, 2 * S), f64)
    for j in range(4):
        for l in range(2):
            h = 2 * j + l
            keyn_blk[l * D:(l + 1) * D, j, l * S:(l + 1) * S] = key_n[h].T
    mvf = mem_value.astype(f64)                                 # [112,512]
    W = np.zeros((P, H, C), f64)
    for h in range(H):
        W[:S, h, :] = mvf @ out_w.astype(f64)[:, h * C:(h + 1) * C].T
    vn = _l2n(mvf, 1)                                           # [112,512]
    mv_pad = np.zeros((P, C), f64)
    mv_pad[:S] = mvf
    blk = np.zeros((P, 4, H), f64)
    for j in range(4):
        for i in range(P):
            blk[i, j, (j * P + i) // D] = 1.0

    def cvt(x, dt):
        return np.ascontiguousarray(np.asarray(x).astype(dt))

    shared = {
        "wqT": cvt(_chunked(q_w.astype(f64).T), qpdt),
        "wvT": cvt(_chunked(v_w.astype(f64).T), qpdt),
        "keyn": cvt(keyn_blk, qpdt),
        "W": cvt(W, BF16),
        "mvpad": cvt(mv_pad, BF16),
        "mvT": cvt(_chunked(mvf.T), qpdt),
        "vnT": cvt(_chunked(vn.T), qpdt),
        "vnT32": cvt(_chunked(vn.T), F32),
        "blk": cvt(blk, qpdt),
        "qb": cvt(q_b.reshape(4, P).T, F32),
        "vb": cvt(v_b.reshape(4, P).T, F32),
    }
    ln1_triv = bool(np.all(ln1_g == 1.0) and np.all(ln1_b == 0.0))
    ln3_triv = bool(np.all(ln3_g == 1.0) and np.all(ln3_b == 0.0))
    ob_triv = bool(np.all(out_b == 0.0))
    if not ln1_triv:
        shared["g1b"] = cvt(np.tile(ln1_g[None, :], (P, 1)), F32)
        shared["b1b"] = cvt(np.tile(ln1_b[None, :], (P, 1)), F32)
    if not ln3_triv:
        shared["g3b"] = cvt(np.tile(ln3_g[None, :], (P, 1)), F32)
        shared["b3b"] = cvt(np.tile(ln3_b[None, :], (P, 1)), F32)
    if not ob_triv:
        shared["obb"] = cvt(np.tile(out_b[None, :], (P, 1)), F32)
    return shared, (ln1_triv, ln3_triv, ob_triv)


def prep_core(q_i, v_i, ntok):
    # q_i, v_i: [ntok, 512] float32
    qpdt = BF16 if QP_BF16 else F32
    iv = 1.0 / np.maximum(np.linalg.norm(v_i.astype(np.float64), axis=1), 1e-12)
    iv = iv / np.sqrt(512.0)
    return {
        "qT": np.ascontiguousarray(q_i.T.astype(qpdt)),
        "q": np.ascontiguousarray(q_i),
        "vT": np.ascontiguousarray(v_i.T.astype(qpdt)),
        "inv_v": np.ascontiguousarray(iv.reshape(ntok // P, P).T.astype(F32)),
    }


def kernel(query, value, mem_key, mem_value, q_w, q_b, v_w, v_b,
           out_w, out_b, ln1_g, ln1_b, ln3_g, ln3_b):
    from concourse.bass_utils import run_bass_kernel_spmd

    query = np.asarray(query, F32)
    value = np.asarray(value, F32)
    B, SEQ, _ = query.shape
    ntok_total = B * SEQ
    ntok = ntok_total // NCORES
    ntiles = ntok // TT

    shared, flags = prep_shared(
        np.asarray(mem_key, F32), np.asarray(mem_value, F32),
        np.asarray(q_w, F32), np.asarray(q_b, F32),
        np.asarray(v_w, F32), np.asarray(v_b, F32),
        np.asarray(out_w, F32), np.asarray(out_b, F32),
        np.asarray(ln1_g, F32), np.asarray(ln1_b, F32),
        np.asarray(ln3_g, F32), np.asarray(ln3_b, F32))

    nc = _get_nc((ntiles,) + flags)

    qs = query.reshape(NCORES, ntok, C)
    vs = value.reshape(NCORES, ntok, C)
    in_maps = []
    for i in range(NCORES):
        m = dict(shared)
        m.update(prep_core(qs[i], vs[i], ntok))
        in_maps.append(m)

    res = run_bass_kernel_spmd(nc, in_maps, list(range(NCORES)))
    global LAST_RESULTS
    LAST_RESULTS = res
    outs = res.results

    f_pred = np.concatenate([outs[i]["f_pred"] for i in range(NCORES)], axis=0)
    f_rec = np.concatenate([outs[i]["f_rec"] for i in range(NCORES)], axis=0)
    recon = sum(float(outs[i]["recon_out"][0, 0]) for i in range(NCORES))
    recon = np.asarray(recon / ntok_total, F32)
    contr = np.asarray(float(outs[0]["contr_out"][0, 0]), F32)
    return (f_pred.reshape(B, SEQ, C), f_rec.reshape(B, SEQ, C), recon, contr)
